# revision 40
# baseline (speedup 1.0000x reference)
"""CrossModalAttention Trainium2 kernel (8-core SPMD, data-parallel over batch).

Self-contained: hardcodes problem shapes (B=16, Tq=Tk=512, D=1024, H=16, hd=64).
kernel(**inputs) takes the full unsharded inputs, shards batches 2-per-core
across 8 NeuronCores, runs a Bass/Tile kernel, and gathers full outputs
(out, attn) matching the reference.
"""

import math
import os
import sys

sys.path.insert(0, "/opt/trn_rl_repo")

import numpy as np
import ml_dtypes

B, T, D, H, HD = 16, 512, 1024, 16, 64
N_CORES = 8
NB = B // N_CORES          # batches per core
KC = D // 128              # feature chunks
MC = T // 128              # token chunks
EPS = 1e-5
NEG = -10000.0
SCALE = 1.0 / math.sqrt(HD)
BF16 = ml_dtypes.bfloat16

MAX_WAITS = 4  # walrus CoreV3 sync-wait slots per instruction

_PROG = None
LAST_EXEC_NS = None
LAST_RESULTS = None


def _patched_tile_context(tile_mod, bass_rust):
    """TileContext adjusted for this walrus build, which allows at most ONE
    sem wait (plus one update) per TPB instruction: excess waits are spilled
    onto standalone EventSemaphore instructions emitted just before the
    over-subscribed instruction on the same engine."""
    ScopedClock = bass_rust.ScopedClock

    def _make_wait_carrier(nc, engine, wait, tag):
        ev = bass_rust.InstEventSemaphore(
            name=f"EVW-{nc.next_id()}-{tag}", ins=[], outs=[]
        )
        ev.engine = engine
        ev.sync_info = bass_rust.SyncInfo(on_wait=[wait], on_update=[])
        return ev

    class TC(tile_mod.TileContext):
        def _lower_ordered_insts(self, ordered):
            nc = self.nc
            for bbname in list(ordered.keys()):
                insts = ordered[bbname]
                new = []
                for inst in insts:
                    si = inst.sync_info
                    waits = list(si.on_wait) if si and si.on_wait else []
                    if len(waits) > 1:
                        for w in waits[:-1]:
                            new.append(
                                _make_wait_carrier(nc, inst.engine, w, inst.name)
                            )
                        inst.sync_info = bass_rust.SyncInfo(
                            on_wait=waits[-1:],
                            on_update=list(si.on_update) if si.on_update else [],
                        )
                    new.append(inst)
                ordered[bbname] = new
            super()._lower_ordered_insts(ordered)

        def _drain_and_barrier(self, tick_clock, wait_clock):
            nc = self.nc
            drain_inst = nc.sync.drain()
            wait_clock.add_sem_waits(
                drain_inst.ins, ScopedClock({None: tick_clock.global_clock})
            )
            si = drain_inst.ins.sync_info
            waits = list(si.on_wait) if si is not None and si.on_wait else []
            if len(waits) > 1:
                drain_inst.ins.sync_info = bass_rust.SyncInfo(
                    on_wait=waits[:1],
                    on_update=list(si.on_update) if si.on_update else [],
                )
                for w in waits[1:]:
                    ev = _make_wait_carrier(nc, drain_inst.ins.engine, w, "drain")
                    nc.engines[ev.engine].add_instruction(ev)
            nc.all_engine_barrier()
            popped = nc._tile_sem_poison_stack.pop()
            assert popped is self._sem_poison
            nc.clear_and_free_semaphores(list(self.sems.allocated().values()))
            nc.all_engine_barrier()

    return TC


def _build_program():
    import concourse.bass as bass
    import concourse.mybir as mybir
    import concourse.tile as tile_mod
    import bass_rust
    from concourse.masks import make_identity

    dt = mybir.dt
    f32, bf16, i32 = dt.float32, dt.bfloat16, dt.int32
    f32r = dt.float32r
    AF = mybir.ActivationFunctionType
    OP = mybir.AluOpType

    nc = bass.Bass("TRN2", target_bir_lowering=False)

    # ---- DRAM I/O (per-core shard) ----
    xq_d = nc.dram_tensor("xq", [NB, T, D], f32, kind="ExternalInput")
    xk_d = nc.dram_tensor("xk", [NB, T, D], f32, kind="ExternalInput")
    xv_d = nc.dram_tensor("xv", [NB, T, D], f32, kind="ExternalInput")
    msk_d = nc.dram_tensor("msk", [NB, T], i32, kind="ExternalInput")
    aq_d = nc.dram_tensor("aq", [D, D], bf16, kind="ExternalInput")
    ak_d = nc.dram_tensor("ak", [D, D], bf16, kind="ExternalInput")
    av_d = nc.dram_tensor("av", [D, D], bf16, kind="ExternalInput")
    ao_d = nc.dram_tensor("ao", [D, D], bf16, kind="ExternalInput")
    cq_d = nc.dram_tensor("cq", [D], f32, kind="ExternalInput")
    ck_d = nc.dram_tensor("ck", [D], f32, kind="ExternalInput")
    cv_d = nc.dram_tensor("cv", [D], f32, kind="ExternalInput")
    co_d = nc.dram_tensor("co", [D], f32, kind="ExternalInput")
    out_d = nc.dram_tensor("out", [NB, T, D], f32, kind="ExternalOutput")
    # attn stored transposed+bf16: [b, h, k, q]; host casts + swaps last axes
    atT_d = nc.dram_tensor("attnT", [NB, H, T, T], bf16, kind="ExternalOutput")
    src_map = {"q": xq_d, "k": xk_d, "v": xv_d}

    TC = _patched_tile_context(tile_mod, bass_rust)

    with TC(nc) as tc:
        with (
            tc.tile_pool(name="wpool", bufs=1) as wpool,
            tc.tile_pool(name="xpool", bufs=3) as xpool,
            tc.tile_pool(name="xnpool", bufs=4) as xnpool,
            tc.tile_pool(name="stpool", bufs=6) as stpool,
            tc.tile_pool(name="xtpool", bufs=2) as xtpool,
            tc.tile_pool(name="qkpool", bufs=2) as qkpool,
            tc.tile_pool(name="vpool", bufs=2) as vpool,
            tc.tile_pool(name="ctpool", bufs=2) as ctpool,
            tc.tile_pool(name="epool", bufs=14) as epool,
            tc.tile_pool(name="zpool", bufs=1) as zpool,
            tc.tile_pool(name="zspool", bufs=4) as zspool,
            tc.tile_pool(name="apool", bufs=8) as apool,
            tc.tile_pool(name="opool", bufs=3) as opool,
            tc.tile_pool(name="mpool", bufs=4) as mpool,
            tc.tile_pool(name="psum", bufs=1, space="PSUM") as psum,
        ):
            # ---- prefetch batch0 'q' activations so LN starts immediately
            # (the sync DMA queues are in-order; weights would block them) ----
            prefetched = {}
            pf = []
            for mc_ in range(MC - 1):
                x_t = xpool.tile([128, D], f32, tag="x", name=f"xpf{mc_}")
                nc.sync.dma_start(out=x_t, in_=xq_d[0, mc_ * 128 : (mc_ + 1) * 128, :])
                pf.append(x_t)
            prefetched[(0, "q")] = pf

            # ---- constants / weights (tiles now; DMAs deferred until after
            # the prologue LN+transpose so activations own the queue first) ----
            aq_s = wpool.tile([128, KC, D], bf16)
            ak_s = wpool.tile([128, KC, D], bf16)
            av_s = wpool.tile([128, KC, D], bf16)
            ao_s = wpool.tile([128, KC, D], bf16)
            cq_s = wpool.tile([128, KC], f32)
            ck_s = wpool.tile([128, KC], f32)

            def emit_weight_dmas():
                nc.sync.dma_start(out=cq_s, in_=cq_d[:].rearrange("(c p) -> p c", p=128))
                aq_r = aq_d[:, :].rearrange("(kc p) n -> p kc n", p=128)
                for nch in range(KC):
                    nc.sync.dma_start(
                        out=aq_s[:, :, nch * 128 : (nch + 1) * 128],
                        in_=aq_r[:, :, nch * 128 : (nch + 1) * 128],
                    )
                nc.gpsimd.dma_start(out=ak_s, in_=ak_d[:, :].rearrange("(kc p) n -> p kc n", p=128))
                nc.gpsimd.dma_start(out=av_s, in_=av_d[:, :].rearrange("(kc p) n -> p kc n", p=128))
                nc.gpsimd.dma_start(out=ao_s, in_=ao_d[:, :].rearrange("(kc p) n -> p kc n", p=128))
                nc.sync.dma_start(out=ck_s, in_=ck_d[:].rearrange("(c p) -> p c", p=128))
            cv_rep = wpool.tile([128, D], bf16)
            nc.gpsimd.dma_start(
                out=cv_rep, in_=bass.AP(tensor=cv_d, offset=0, ap=[[0, 128], [1, D]])
            )
            co_rep = wpool.tile([128, D], bf16)
            nc.gpsimd.dma_start(
                out=co_rep, in_=bass.AP(tensor=co_d, offset=0, ap=[[0, 128], [1, D]])
            )
            idn = wpool.tile([128, 128], bf16)
            make_identity(nc, idn)
            ones_col = wpool.tile([1, 128], f32r)
            nc.vector.memset(ones_col.bitcast(f32), 1.0)
            eps_t = wpool.tile([128, 1], f32)
            nc.vector.memset(eps_t, EPS)

            st_ = {}  # per-batch live tiles: mb, qt, kt, vt, ct

            def emit_mask(b):
                mi = mpool.tile([128, MC], i32, name=f"mi{b}")
                nc.sync.dma_start(out=mi, in_=msk_d[b, :].rearrange("(c p) -> p c", p=128))
                mf = mpool.tile([128, MC], f32, name=f"mf{b}")
                nc.vector.tensor_copy(out=mf, in_=mi)
                mb_t = mpool.tile([128, MC], f32, name=f"mb{b}")
                nc.vector.tensor_scalar(
                    out=mb_t, in0=mf, scalar1=-NEG, scalar2=NEG, op0=OP.mult, op1=OP.add
                )
                st_[b] = {"mb": mb_t}

            def emit_lnt(b, tname, xn_on_act):
                """LN + PE-transpose one input tensor -> feature-major bf16."""
                xn_tiles = []
                pre = prefetched.pop((b, tname), None)
                for mc_ in range(MC):
                    if pre is not None and mc_ < len(pre):
                        x_t = pre[mc_]
                    else:
                        x_t = xpool.tile([128, D], f32, tag="x", name="x")
                        nc.scalar.dma_start(
                            out=x_t, in_=src_map[tname][b, mc_ * 128 : (mc_ + 1) * 128, :]
                        )
                    stt = stpool.tile([128, 2, 6], f32, tag="st", name="stt")
                    nc.vector.bn_stats(out=stt[:, 0, :], in_=x_t[:, 0:512])
                    nc.vector.bn_stats(out=stt[:, 1, :], in_=x_t[:, 512:1024])
                    mv = stpool.tile([128, 2], f32, tag="mv", name="mv")
                    nc.vector.bn_aggr(out=mv, in_=stt)
                    # rstd = exp(-0.5 * ln(var + eps)) (stays in exp table set)
                    lnv = stpool.tile([128, 1], f32, tag="lnv", name="lnv")
                    nc.scalar.activation(
                        out=lnv, in_=mv[:, 1:2], func=AF.Ln, bias=eps_t, scale=1.0
                    )
                    r_t = stpool.tile([128, 1], f32, tag="r", name="r_t")
                    nc.scalar.activation(out=r_t, in_=lnv, func=AF.Exp, scale=-0.5)
                    xn_t = xnpool.tile([128, D], bf16, tag="xn", name="xn")
                    if xn_on_act:
                        nmu = stpool.tile([128, 1], f32, tag="nmu", name="nmu")
                        nc.vector.scalar_tensor_tensor(
                            out=nmu, in0=mv[:, 0:1], scalar=-1.0, in1=r_t,
                            op0=OP.mult, op1=OP.mult,
                        )
                        nc.scalar.activation(
                            out=xn_t, in_=x_t, func=AF.Identity, bias=nmu, scale=r_t
                        )
                    else:
                        nmu = stpool.tile([128, 1], f32, tag="nmu", name="nmu")
                        nc.vector.tensor_scalar_mul(out=nmu, in0=mv[:, 0:1], scalar1=-1.0)
                        nc.vector.tensor_scalar(
                            out=xn_t, in0=x_t, scalar1=nmu, scalar2=r_t,
                            op0=OP.add, op1=OP.mult,
                        )
                    xn_tiles.append(xn_t)
                xt = xtpool.tile([128, KC, T], bf16, tag="xt", name=f"xt{b}{tname}")
                for kc_ in range(KC):
                    tp = psum.tile([128, 512], bf16, tag="sc", bufs=3, name="tp")
                    for mc_ in range(MC):
                        nc.tensor.transpose(
                            out=tp[:, mc_ * 128 : (mc_ + 1) * 128],
                            in_=xn_tiles[mc_][:, kc_ * 128 : (kc_ + 1) * 128],
                            identity=idn,
                        )
                    nc.scalar.copy(out=xt[:, kc_, :], in_=tp)
                return xt

            def emit_projqk(b, tname, xt):
                w_s, c_s = (aq_s, cq_s) if tname == "q" else (ak_s, ck_s)
                dst = qkpool.tile(
                    [128, KC, T], bf16, tag="qt" if tname == "q" else "kt",
                    name=f"{tname}t{b}",
                )
                st_[b]["qt" if tname == "q" else "kt"] = dst
                for nch in range(KC):
                    pj = psum.tile([128, 512], f32, tag="pj", bufs=3, name="pj")
                    for kc_ in range(KC):
                        nc.tensor.matmul(
                            pj,
                            lhsT=w_s[:, kc_, nch * 128 : (nch + 1) * 128],
                            rhs=xt[:, kc_, :],
                            start=(kc_ == 0),
                            stop=(kc_ == KC - 1),
                        )
                    nc.vector.tensor_scalar_add(
                        out=dst[:, nch, :], in0=pj, scalar1=c_s[:, nch : nch + 1]
                    )

            def emit_projv(b, xt, mc_list):
                if "vt" not in st_[b]:
                    vt = vpool.tile([128, MC, H, HD + 1], bf16, tag="vt", name=f"vt{b}")
                    nc.vector.memset(vt[:, :, :, HD : HD + 1], 1.0)
                    st_[b]["vt"] = vt
                vt = st_[b]["vt"]
                for mc_ in mc_list:
                    for nh in range(2):
                        pj = psum.tile([128, 512], f32, tag="pj", bufs=3, name="pj")
                        for kc_ in range(KC):
                            nc.tensor.matmul(
                                pj,
                                lhsT=xt[:, kc_, mc_ * 128 : (mc_ + 1) * 128],
                                rhs=av_s[:, kc_, nh * 512 : (nh + 1) * 512],
                                start=(kc_ == 0),
                                stop=(kc_ == KC - 1),
                            )
                        nc.vector.tensor_tensor(
                            out=vt[:, mc_, nh * 8 : (nh + 1) * 8, 0:HD],
                            in0=pj.rearrange("p (h d) -> p h d", d=HD),
                            in1=cv_rep[:, nh * 512 : (nh + 1) * 512].rearrange(
                                "p (h d) -> p h d", d=HD
                            ),
                            op=OP.add,
                        )

            pending_norms = []

            def flush_stores():
                # previous pair's zs/e tiles are ready; normalize + store now,
                # off the inter-pair critical path
                for dve_side, e_t, zs, dst in pending_norms:
                    at = apool.tile([128, 512], bf16, tag="at", name="at")
                    eng = nc.vector if dve_side else nc.gpsimd
                    eng.tensor_tensor(out=at, in0=e_t, in1=zs, op=OP.mult)
                    nc.sync.dma_start(out=dst, in_=at)
                pending_norms.clear()

            def emit_attn_pair(b, pr, filler=None):
                """Two heads; STs+exps first, then filler (next batch's PE
                work) runs while ACT exps, then AVs find exps done."""
                qt, kt, vt, mb_t = st_[b]["qt"], st_[b]["kt"], st_[b]["vt"], st_[b]["mb"]
                if "ct" not in st_[b]:
                    st_[b]["ct"] = ctpool.tile([128, KC, T], bf16, tag="ct", name=f"ct{b}")
                ct = st_[b]["ct"]
                ctzs = [
                    psum.tile([HD + 1, 512], f32, tag="ctz", bufs=2, name="ctza"),
                    psum.tile([HD + 1, 512], f32, tag="ctz", bufs=2, name="ctzb"),
                ]
                e_ts = [[], []]
                for c in range(MC):
                    stps = []
                    for hh in range(2):
                        off = hh * HD
                        stp = psum.tile([128, 512], f32, tag="sc", bufs=3, name="stp")
                        nc.tensor.matmul(
                            stp,
                            lhsT=kt[off : off + HD, pr, c * 128 : (c + 1) * 128],
                            rhs=qt[off : off + HD, pr, :],
                            start=True,
                            stop=True,
                            tile_position=(off, 0),
                        )
                        stps.append(stp)
                    for hh in range(2):
                        e_t = epool.tile([128, 512], bf16, tag="e", name="e")
                        nc.scalar.activation(
                            out=e_t, in_=stps[hh], func=AF.Exp,
                            bias=mb_t[:, c : c + 1], scale=SCALE,
                        )
                        e_ts[hh].append(e_t)
                if filler is not None:
                    filler()
                for c in range(MC):
                    for hh in range(2):
                        nc.tensor.matmul(
                            ctzs[hh], lhsT=vt[:, c, 2 * pr + hh, :], rhs=e_ts[hh][c],
                            start=(c == 0), stop=(c == MC - 1),
                        )
                for hh in range(2):
                    h = 2 * pr + hh
                    off = hh * HD
                    ctz = ctzs[hh]
                    # 1/Z = exp(-ln(Z)) on ACT (same table set as Exp)
                    zln = zpool.tile([1, 512], f32, tag="zln", name="zln")
                    nc.scalar.activation(out=zln, in_=ctz[HD : HD + 1, :], func=AF.Ln)
                    rz = zpool.tile([1, 512], f32r, tag="rz", name="rz")
                    nc.scalar.activation(out=rz, in_=zln, func=AF.Exp, scale=-1.0)
                    zrep = psum.tile([128, 512], f32, tag="sc", bufs=3, name="zrep")
                    nc.tensor.matmul(zrep, lhsT=ones_col, rhs=rz, start=True, stop=True)
                    zs = zspool.tile([128, 512], bf16, tag="zs", name="zs")
                    nc.vector.tensor_copy(out=zs, in_=zrep)
                    nc.vector.tensor_tensor(
                        out=ct[off : off + HD, pr, :], in0=ctz[0:HD, :],
                        in1=zs[0:HD, :], op=OP.mult,
                    )
                    for c in range(MC):
                        dve_side = (c % 2 == hh % 2)
                        pending_norms.append(
                            (dve_side, e_ts[hh][c], zs,
                             atT_d[b, h, c * 128 : (c + 1) * 128, :])
                        )

            def emit_outproj_half(b, mc_, nh):
                ct = st_[b]["ct"]
                pj = psum.tile([128, 512], f32, tag="pj", bufs=3, name="pjo")
                for j in range(KC):
                    nc.tensor.matmul(
                        pj,
                        lhsT=ct[:, j, mc_ * 128 : (mc_ + 1) * 128],
                        rhs=ao_s[:, j, nh * 512 : (nh + 1) * 512],
                        start=(j == 0),
                        stop=(j == KC - 1),
                    )
                ot = opool.tile([128, 512], f32, tag="ot", name="ot")
                nc.vector.tensor_tensor(
                    out=ot, in0=pj,
                    in1=co_rep[:, nh * 512 : (nh + 1) * 512], op=OP.add,
                )
                nc.sync.dma_start(
                    out=out_d[b, mc_ * 128 : (mc_ + 1) * 128,
                              nh * 512 : (nh + 1) * 512],
                    in_=ot,
                )

            def emit_outproj(b, mc_list):
                for mc_ in mc_list:
                    for nh in range(2):
                        emit_outproj_half(b, mc_, nh)

            # ---- schedule: prologue batch0, then attn(b) interleaved with
            # batch b+1 projection pipeline / batch b-1 out-projection ----
            emit_mask(0)
            xt_q0 = emit_lnt(0, "q", xn_on_act=True)
            emit_weight_dmas()
            emit_projqk(0, "q", xt_q0)
            xt_k0 = emit_lnt(0, "k", xn_on_act=True)
            emit_projqk(0, "k", xt_k0)
            xt_v0 = emit_lnt(0, "v", xn_on_act=True)
            emit_projv(0, xt_v0, [0, 1, 2, 3])

            fillers_b1 = [
                lambda: emit_mask(1),
                lambda: fill_lnt(1, "q"),
                lambda: fill_proj(1, "q"),
                lambda: fill_lnt(1, "k"),
                lambda: fill_proj(1, "k"),
                lambda: fill_lnt(1, "v"),
                lambda: fill_projv(1, [0, 1]),
                lambda: fill_projv(1, [2, 3]),
            ]
            _xts = {}

            def fill_lnt(b, tname):
                _xts[(b, tname)] = emit_lnt(b, tname, xn_on_act=False)

            def fill_proj(b, tname):
                emit_projqk(b, tname, _xts.pop((b, tname)))

            def fill_projv(b, mc_list):
                emit_projv(b, _xts[(b, "v")] if (b, "v") in _xts else None, mc_list)

            def fill_projv(b, mc_list):  # noqa: F811
                emit_projv(b, _xts[(b, "v")], mc_list)
                if mc_list[-1] == 3:
                    _xts.pop((b, "v"))

            for pr in range(H // 2):
                flush_stores()
                emit_attn_pair(0, pr, filler=fillers_b1[pr])

            fillers_o0 = [
                (lambda m=m, n=n: emit_outproj_half(0, m, n))
                for m in range(MC) for n in range(2)
            ]
            for pr in range(H // 2):
                flush_stores()
                emit_attn_pair(1, pr, filler=fillers_o0[pr])
            flush_stores()
            emit_outproj(1, [0, 1, 2, 3])

    return nc


def get_program():
    global _PROG
    if _PROG is None:
        _PROG = _build_program()
    return _PROG


def _host_prep(query, key, value, mask, q_ln_g, q_ln_b, k_ln_g, k_ln_b,
               v_ln_g, v_ln_b, Wq, bq, Wk, bk, Wv, bv, Wo, bo):
    """Fold LN affine into projection weights; pre-transpose; cast to bf16."""
    f = np.float32
    Aq = np.ascontiguousarray((Wq.astype(f) * q_ln_g.astype(f)[None, :]).T).astype(BF16)
    Ak = np.ascontiguousarray((Wk.astype(f) * k_ln_g.astype(f)[None, :]).T).astype(BF16)
    Av = np.ascontiguousarray((Wv.astype(f) * v_ln_g.astype(f)[None, :]).T).astype(BF16)
    Ao = np.ascontiguousarray(Wo.astype(f).T).astype(BF16)
    cq = (bq.astype(f) + Wq.astype(f) @ q_ln_b.astype(f)).astype(f)
    ck = (bk.astype(f) + Wk.astype(f) @ k_ln_b.astype(f)).astype(f)
    cv = (bv.astype(f) + Wv.astype(f) @ v_ln_b.astype(f)).astype(f)
    co = bo.astype(f)
    return Aq, Ak, Av, Ao, cq, ck, cv, co


def kernel(query, key, value, mask, q_ln_g, q_ln_b, k_ln_g, k_ln_b,
           v_ln_g, v_ln_b, Wq, bq, Wk, bk, Wv, bv, Wo, bo):
    global LAST_EXEC_NS, LAST_RESULTS
    from concourse.bass_utils import run_bass_kernel_spmd

    nc = get_program()
    Aq, Ak, Av, Ao, cq, ck, cv, co = _host_prep(
        query, key, value, mask, q_ln_g, q_ln_b, k_ln_g, k_ln_b,
        v_ln_g, v_ln_b, Wq, bq, Wk, bk, Wv, bv, Wo, bo,
    )
    query = np.ascontiguousarray(np.asarray(query, np.float32))
    key = np.ascontiguousarray(np.asarray(key, np.float32))
    value = np.ascontiguousarray(np.asarray(value, np.float32))
    mask = np.ascontiguousarray(np.asarray(mask, np.int32))

    in_maps = []
    for c in range(N_CORES):
        sl = slice(c * NB, (c + 1) * NB)
        in_maps.append({
            "xq": query[sl], "xk": key[sl], "xv": value[sl], "msk": mask[sl],
            "aq": Aq, "ak": Ak, "av": Av, "ao": Ao,
            "cq": cq, "ck": ck, "cv": cv, "co": co,
        })

    trace = bool(int(os.environ.get("ATTN_TRACE", "0")))
    tmpdir = os.environ.get("ATTN_TRACE_DIR") or None
    res = run_bass_kernel_spmd(
        nc, in_maps, list(range(N_CORES)), trace=trace, tmpdir=tmpdir
    )
    LAST_EXEC_NS = res.exec_time_ns
    LAST_RESULTS = res

    out = np.concatenate([r["out"] for r in res.results], axis=0)
    attnT = np.concatenate([r["attnT"] for r in res.results], axis=0)
    attn = np.ascontiguousarray(attnT.transpose(0, 1, 3, 2).astype(np.float32))
    return out, attn


# revision 41
# speedup vs baseline: 1.0183x; 1.0183x over previous
"""CrossModalAttention Trainium2 kernel (8-core SPMD, data-parallel over batch).

Self-contained: hardcodes problem shapes (B=16, Tq=Tk=512, D=1024, H=16, hd=64).
kernel(**inputs) takes the full unsharded inputs, shards batches 2-per-core
across 8 NeuronCores, runs a Bass/Tile kernel, and gathers full outputs
(out, attn) matching the reference.
"""

import math
import os
import sys

sys.path.insert(0, "/opt/trn_rl_repo")

import numpy as np
import ml_dtypes

B, T, D, H, HD = 16, 512, 1024, 16, 64
N_CORES = 8
NB = B // N_CORES          # batches per core
KC = D // 128              # feature chunks
MC = T // 128              # token chunks
EPS = 1e-5
NEG = -10000.0
SCALE = 1.0 / math.sqrt(HD)
BF16 = ml_dtypes.bfloat16

MAX_WAITS = 4  # walrus CoreV3 sync-wait slots per instruction

_PROG = None
LAST_EXEC_NS = None
LAST_RESULTS = None


def _patched_tile_context(tile_mod, bass_rust):
    """TileContext adjusted for this walrus build, which allows at most ONE
    sem wait (plus one update) per TPB instruction: excess waits are spilled
    onto standalone EventSemaphore instructions emitted just before the
    over-subscribed instruction on the same engine."""
    ScopedClock = bass_rust.ScopedClock

    def _make_wait_carrier(nc, engine, wait, tag):
        ev = bass_rust.InstEventSemaphore(
            name=f"EVW-{nc.next_id()}-{tag}", ins=[], outs=[]
        )
        ev.engine = engine
        ev.sync_info = bass_rust.SyncInfo(on_wait=[wait], on_update=[])
        return ev

    class TC(tile_mod.TileContext):
        def _lower_ordered_insts(self, ordered):
            nc = self.nc
            for bbname in list(ordered.keys()):
                insts = ordered[bbname]
                new = []
                for inst in insts:
                    si = inst.sync_info
                    waits = list(si.on_wait) if si and si.on_wait else []
                    if len(waits) > 1:
                        for w in waits[:-1]:
                            new.append(
                                _make_wait_carrier(nc, inst.engine, w, inst.name)
                            )
                        inst.sync_info = bass_rust.SyncInfo(
                            on_wait=waits[-1:],
                            on_update=list(si.on_update) if si.on_update else [],
                        )
                    new.append(inst)
                ordered[bbname] = new
            super()._lower_ordered_insts(ordered)

        def _drain_and_barrier(self, tick_clock, wait_clock):
            nc = self.nc
            drain_inst = nc.sync.drain()
            wait_clock.add_sem_waits(
                drain_inst.ins, ScopedClock({None: tick_clock.global_clock})
            )
            si = drain_inst.ins.sync_info
            waits = list(si.on_wait) if si is not None and si.on_wait else []
            if len(waits) > 1:
                drain_inst.ins.sync_info = bass_rust.SyncInfo(
                    on_wait=waits[:1],
                    on_update=list(si.on_update) if si.on_update else [],
                )
                for w in waits[1:]:
                    ev = _make_wait_carrier(nc, drain_inst.ins.engine, w, "drain")
                    nc.engines[ev.engine].add_instruction(ev)
            nc.all_engine_barrier()
            popped = nc._tile_sem_poison_stack.pop()
            assert popped is self._sem_poison
            nc.clear_and_free_semaphores(list(self.sems.allocated().values()))
            nc.all_engine_barrier()

    return TC


def _build_program():
    import concourse.bass as bass
    import concourse.mybir as mybir
    import concourse.tile as tile_mod
    import bass_rust
    from concourse.masks import make_identity

    dt = mybir.dt
    f32, bf16, i32 = dt.float32, dt.bfloat16, dt.int32
    f32r = dt.float32r
    AF = mybir.ActivationFunctionType
    OP = mybir.AluOpType

    nc = bass.Bass("TRN2", target_bir_lowering=False)

    # ---- DRAM I/O (per-core shard) ----
    xq_d = nc.dram_tensor("xq", [NB, T, D], f32, kind="ExternalInput")
    xk_d = nc.dram_tensor("xk", [NB, T, D], f32, kind="ExternalInput")
    xv_d = nc.dram_tensor("xv", [NB, T, D], f32, kind="ExternalInput")
    msk_d = nc.dram_tensor("msk", [NB, T], i32, kind="ExternalInput")
    aq_d = nc.dram_tensor("aq", [D, D], bf16, kind="ExternalInput")
    ak_d = nc.dram_tensor("ak", [D, D], bf16, kind="ExternalInput")
    av_d = nc.dram_tensor("av", [D, D], bf16, kind="ExternalInput")
    ao_d = nc.dram_tensor("ao", [D, D], bf16, kind="ExternalInput")
    cq_d = nc.dram_tensor("cq", [D], f32, kind="ExternalInput")
    ck_d = nc.dram_tensor("ck", [D], f32, kind="ExternalInput")
    cv_d = nc.dram_tensor("cv", [D], f32, kind="ExternalInput")
    co_d = nc.dram_tensor("co", [D], f32, kind="ExternalInput")
    out_d = nc.dram_tensor("out", [NB, T, D], f32, kind="ExternalOutput")
    # attn stored transposed+bf16: [b, h, k, q]; host casts + swaps last axes
    atT_d = nc.dram_tensor("attnT", [NB, H, T, T], bf16, kind="ExternalOutput")
    src_map = {"q": xq_d, "k": xk_d, "v": xv_d}

    TC = _patched_tile_context(tile_mod, bass_rust)

    with TC(nc) as tc:
        with (
            tc.tile_pool(name="wpool", bufs=1) as wpool,
            tc.tile_pool(name="xpool", bufs=3) as xpool,
            tc.tile_pool(name="xnpool", bufs=4) as xnpool,
            tc.tile_pool(name="stpool", bufs=6) as stpool,
            tc.tile_pool(name="xtpool", bufs=2) as xtpool,
            tc.tile_pool(name="qkpool", bufs=2) as qkpool,
            tc.tile_pool(name="vpool", bufs=2) as vpool,
            tc.tile_pool(name="ctpool", bufs=2) as ctpool,
            tc.tile_pool(name="epool", bufs=14) as epool,
            tc.tile_pool(name="zpool", bufs=1) as zpool,
            tc.tile_pool(name="zspool", bufs=4) as zspool,
            tc.tile_pool(name="apool", bufs=8) as apool,
            tc.tile_pool(name="opool", bufs=3) as opool,
            tc.tile_pool(name="mpool", bufs=4) as mpool,
            tc.tile_pool(name="psum", bufs=1, space="PSUM") as psum,
        ):
            # ---- prefetch batch0 'q' activations so LN starts immediately
            # (the sync DMA queues are in-order; weights would block them) ----
            prefetched = {}
            pf = []
            for mc_ in range(MC - 1):
                x_t = xpool.tile([128, D], f32, tag="x", name=f"xpf{mc_}")
                nc.sync.dma_start(out=x_t, in_=xq_d[0, mc_ * 128 : (mc_ + 1) * 128, :])
                pf.append(x_t)
            prefetched[(0, "q")] = pf

            # ---- constants / weights (tiles now; DMAs deferred until after
            # the prologue LN+transpose so activations own the queue first) ----
            aq_s = wpool.tile([128, KC, D], bf16)
            ak_s = wpool.tile([128, KC, D], bf16)
            av_s = wpool.tile([128, KC, D], bf16)
            ao_s = wpool.tile([128, KC, D], bf16)
            cq_s = wpool.tile([128, KC], f32)
            ck_s = wpool.tile([128, KC], f32)

            def emit_weight_dmas():
                nc.sync.dma_start(out=cq_s, in_=cq_d[:].rearrange("(c p) -> p c", p=128))
                aq_r = aq_d[:, :].rearrange("(kc p) n -> p kc n", p=128)
                for qtr in range(4):
                    nc.sync.dma_start(
                        out=aq_s[:, 2 * qtr : 2 * qtr + 2, :],
                        in_=aq_r[:, 2 * qtr : 2 * qtr + 2, :],
                    )
                nc.gpsimd.dma_start(out=ak_s, in_=ak_d[:, :].rearrange("(kc p) n -> p kc n", p=128))
                nc.gpsimd.dma_start(out=av_s, in_=av_d[:, :].rearrange("(kc p) n -> p kc n", p=128))
                nc.gpsimd.dma_start(out=ao_s, in_=ao_d[:, :].rearrange("(kc p) n -> p kc n", p=128))
                nc.sync.dma_start(out=ck_s, in_=ck_d[:].rearrange("(c p) -> p c", p=128))
            cv_rep = wpool.tile([128, D], bf16)
            nc.gpsimd.dma_start(
                out=cv_rep, in_=bass.AP(tensor=cv_d, offset=0, ap=[[0, 128], [1, D]])
            )
            co_rep = wpool.tile([128, D], bf16)
            nc.gpsimd.dma_start(
                out=co_rep, in_=bass.AP(tensor=co_d, offset=0, ap=[[0, 128], [1, D]])
            )
            idn = wpool.tile([128, 128], bf16)
            make_identity(nc, idn)
            ones_col = wpool.tile([1, 128], f32r)
            nc.vector.memset(ones_col.bitcast(f32), 1.0)
            eps_t = wpool.tile([128, 1], f32)
            nc.vector.memset(eps_t, EPS)

            st_ = {}  # per-batch live tiles: mb, qt, kt, vt, ct

            def emit_mask(b):
                mi = mpool.tile([128, MC], i32, name=f"mi{b}")
                nc.sync.dma_start(out=mi, in_=msk_d[b, :].rearrange("(c p) -> p c", p=128))
                mf = mpool.tile([128, MC], f32, name=f"mf{b}")
                nc.vector.tensor_copy(out=mf, in_=mi)
                mb_t = mpool.tile([128, MC], f32, name=f"mb{b}")
                nc.vector.tensor_scalar(
                    out=mb_t, in0=mf, scalar1=-NEG, scalar2=NEG, op0=OP.mult, op1=OP.add
                )
                st_[b] = {"mb": mb_t}

            def emit_lnt(b, tname, xn_on_act):
                """LN + PE-transpose one input tensor -> feature-major bf16."""
                xn_tiles = []
                pre = prefetched.pop((b, tname), None)
                for mc_ in range(MC):
                    if pre is not None and mc_ < len(pre):
                        x_t = pre[mc_]
                    else:
                        x_t = xpool.tile([128, D], f32, tag="x", name="x")
                        nc.scalar.dma_start(
                            out=x_t, in_=src_map[tname][b, mc_ * 128 : (mc_ + 1) * 128, :]
                        )
                    stt = stpool.tile([128, 2, 6], f32, tag="st", name="stt")
                    nc.vector.bn_stats(out=stt[:, 0, :], in_=x_t[:, 0:512])
                    nc.vector.bn_stats(out=stt[:, 1, :], in_=x_t[:, 512:1024])
                    mv = stpool.tile([128, 2], f32, tag="mv", name="mv")
                    nc.vector.bn_aggr(out=mv, in_=stt)
                    # rstd = exp(-0.5 * ln(var + eps)) (stays in exp table set)
                    lnv = stpool.tile([128, 1], f32, tag="lnv", name="lnv")
                    nc.scalar.activation(
                        out=lnv, in_=mv[:, 1:2], func=AF.Ln, bias=eps_t, scale=1.0
                    )
                    r_t = stpool.tile([128, 1], f32, tag="r", name="r_t")
                    nc.scalar.activation(out=r_t, in_=lnv, func=AF.Exp, scale=-0.5)
                    xn_t = xnpool.tile([128, D], bf16, tag="xn", name="xn")
                    if xn_on_act:
                        nmu = stpool.tile([128, 1], f32, tag="nmu", name="nmu")
                        nc.vector.scalar_tensor_tensor(
                            out=nmu, in0=mv[:, 0:1], scalar=-1.0, in1=r_t,
                            op0=OP.mult, op1=OP.mult,
                        )
                        nc.scalar.activation(
                            out=xn_t, in_=x_t, func=AF.Identity, bias=nmu, scale=r_t
                        )
                    else:
                        nmu = stpool.tile([128, 1], f32, tag="nmu", name="nmu")
                        nc.vector.tensor_scalar_mul(out=nmu, in0=mv[:, 0:1], scalar1=-1.0)
                        nc.vector.tensor_scalar(
                            out=xn_t, in0=x_t, scalar1=nmu, scalar2=r_t,
                            op0=OP.add, op1=OP.mult,
                        )
                    xn_tiles.append(xn_t)
                xt = xtpool.tile([128, KC, T], bf16, tag="xt", name=f"xt{b}{tname}")
                for kc_ in range(KC):
                    tp = psum.tile([128, 512], bf16, tag="sc", bufs=3, name="tp")
                    for mc_ in range(MC):
                        nc.tensor.transpose(
                            out=tp[:, mc_ * 128 : (mc_ + 1) * 128],
                            in_=xn_tiles[mc_][:, kc_ * 128 : (kc_ + 1) * 128],
                            identity=idn,
                        )
                    nc.scalar.copy(out=xt[:, kc_, :], in_=tp)
                return xt

            def emit_projqk(b, tname, xt):
                w_s, c_s = (aq_s, cq_s) if tname == "q" else (ak_s, ck_s)
                dst = qkpool.tile(
                    [128, KC, T], bf16, tag="qt" if tname == "q" else "kt",
                    name=f"{tname}t{b}",
                )
                st_[b]["qt" if tname == "q" else "kt"] = dst
                for nch in range(KC):
                    pj = psum.tile([128, 512], f32, tag="pj", bufs=3, name="pj")
                    for kc_ in range(KC):
                        nc.tensor.matmul(
                            pj,
                            lhsT=w_s[:, kc_, nch * 128 : (nch + 1) * 128],
                            rhs=xt[:, kc_, :],
                            start=(kc_ == 0),
                            stop=(kc_ == KC - 1),
                        )
                    nc.vector.tensor_scalar_add(
                        out=dst[:, nch, :], in0=pj, scalar1=c_s[:, nch : nch + 1]
                    )

            def emit_projv(b, xt, mc_list):
                if "vt" not in st_[b]:
                    vt = vpool.tile([128, MC, H, HD + 1], bf16, tag="vt", name=f"vt{b}")
                    nc.vector.memset(vt[:, :, :, HD : HD + 1], 1.0)
                    st_[b]["vt"] = vt
                vt = st_[b]["vt"]
                for mc_ in mc_list:
                    for nh in range(2):
                        pj = psum.tile([128, 512], f32, tag="pj", bufs=3, name="pj")
                        for kc_ in range(KC):
                            nc.tensor.matmul(
                                pj,
                                lhsT=xt[:, kc_, mc_ * 128 : (mc_ + 1) * 128],
                                rhs=av_s[:, kc_, nh * 512 : (nh + 1) * 512],
                                start=(kc_ == 0),
                                stop=(kc_ == KC - 1),
                            )
                        nc.vector.tensor_tensor(
                            out=vt[:, mc_, nh * 8 : (nh + 1) * 8, 0:HD],
                            in0=pj.rearrange("p (h d) -> p h d", d=HD),
                            in1=cv_rep[:, nh * 512 : (nh + 1) * 512].rearrange(
                                "p (h d) -> p h d", d=HD
                            ),
                            op=OP.add,
                        )

            pending_norms = []

            def flush_stores():
                # previous pair's zs/e tiles are ready; normalize + store now,
                # off the inter-pair critical path
                for dve_side, e_t, zs, dst in pending_norms:
                    at = apool.tile([128, 512], bf16, tag="at", name="at")
                    eng = nc.vector if dve_side else nc.gpsimd
                    eng.tensor_tensor(out=at, in0=e_t, in1=zs, op=OP.mult)
                    nc.sync.dma_start(out=dst, in_=at)
                pending_norms.clear()

            def emit_attn_pair(b, pr, filler=None):
                """Two heads; STs+exps first, then filler (next batch's PE
                work) runs while ACT exps, then AVs find exps done."""
                qt, kt, vt, mb_t = st_[b]["qt"], st_[b]["kt"], st_[b]["vt"], st_[b]["mb"]
                if "ct" not in st_[b]:
                    st_[b]["ct"] = ctpool.tile([128, KC, T], bf16, tag="ct", name=f"ct{b}")
                ct = st_[b]["ct"]
                ctzs = [
                    psum.tile([HD + 1, 512], f32, tag="ctz", bufs=2, name="ctza"),
                    psum.tile([HD + 1, 512], f32, tag="ctz", bufs=2, name="ctzb"),
                ]
                e_ts = [[], []]
                for c in range(MC):
                    stps = []
                    for hh in range(2):
                        off = hh * HD
                        stp = psum.tile([128, 512], f32, tag="sc", bufs=3, name="stp")
                        nc.tensor.matmul(
                            stp,
                            lhsT=kt[off : off + HD, pr, c * 128 : (c + 1) * 128],
                            rhs=qt[off : off + HD, pr, :],
                            start=True,
                            stop=True,
                            tile_position=(off, 0),
                        )
                        stps.append(stp)
                    for hh in range(2):
                        e_t = epool.tile([128, 512], bf16, tag="e", name="e")
                        nc.scalar.activation(
                            out=e_t, in_=stps[hh], func=AF.Exp,
                            bias=mb_t[:, c : c + 1], scale=SCALE,
                        )
                        e_ts[hh].append(e_t)
                if filler is not None:
                    filler()
                for c in range(MC):
                    for hh in range(2):
                        nc.tensor.matmul(
                            ctzs[hh], lhsT=vt[:, c, 2 * pr + hh, :], rhs=e_ts[hh][c],
                            start=(c == 0), stop=(c == MC - 1),
                        )
                for hh in range(2):
                    h = 2 * pr + hh
                    off = hh * HD
                    ctz = ctzs[hh]
                    # 1/Z = exp(-ln(Z)) on ACT (same table set as Exp)
                    zln = zpool.tile([1, 512], f32, tag="zln", name="zln")
                    nc.scalar.activation(out=zln, in_=ctz[HD : HD + 1, :], func=AF.Ln)
                    rz = zpool.tile([1, 512], f32r, tag="rz", name="rz")
                    nc.scalar.activation(out=rz, in_=zln, func=AF.Exp, scale=-1.0)
                    zrep = psum.tile([128, 512], f32, tag="sc", bufs=3, name="zrep")
                    nc.tensor.matmul(zrep, lhsT=ones_col, rhs=rz, start=True, stop=True)
                    zs = zspool.tile([128, 512], bf16, tag="zs", name="zs")
                    nc.vector.tensor_copy(out=zs, in_=zrep)
                    nc.vector.tensor_tensor(
                        out=ct[off : off + HD, pr, :], in0=ctz[0:HD, :],
                        in1=zs[0:HD, :], op=OP.mult,
                    )
                    for c in range(MC):
                        dve_side = (c % 2 == hh % 2)
                        pending_norms.append(
                            (dve_side, e_ts[hh][c], zs,
                             atT_d[b, h, c * 128 : (c + 1) * 128, :])
                        )

            def emit_outproj_half(b, mc_, nh):
                ct = st_[b]["ct"]
                pj = psum.tile([128, 512], f32, tag="pj", bufs=3, name="pjo")
                for j in range(KC):
                    nc.tensor.matmul(
                        pj,
                        lhsT=ct[:, j, mc_ * 128 : (mc_ + 1) * 128],
                        rhs=ao_s[:, j, nh * 512 : (nh + 1) * 512],
                        start=(j == 0),
                        stop=(j == KC - 1),
                    )
                ot = opool.tile([128, 512], f32, tag="ot", name="ot")
                nc.vector.tensor_tensor(
                    out=ot, in0=pj,
                    in1=co_rep[:, nh * 512 : (nh + 1) * 512], op=OP.add,
                )
                nc.sync.dma_start(
                    out=out_d[b, mc_ * 128 : (mc_ + 1) * 128,
                              nh * 512 : (nh + 1) * 512],
                    in_=ot,
                )

            def emit_outproj(b, mc_list):
                for mc_ in mc_list:
                    for nh in range(2):
                        emit_outproj_half(b, mc_, nh)

            # ---- schedule: prologue batch0, then attn(b) interleaved with
            # batch b+1 projection pipeline / batch b-1 out-projection ----
            emit_mask(0)
            xt_q0 = emit_lnt(0, "q", xn_on_act=True)
            emit_weight_dmas()
            emit_projqk(0, "q", xt_q0)
            xt_k0 = emit_lnt(0, "k", xn_on_act=True)
            emit_projqk(0, "k", xt_k0)
            xt_v0 = emit_lnt(0, "v", xn_on_act=True)
            emit_projv(0, xt_v0, [0, 1, 2, 3])

            fillers_b1 = [
                lambda: emit_mask(1),
                lambda: fill_lnt(1, "q"),
                lambda: fill_proj(1, "q"),
                lambda: fill_lnt(1, "k"),
                lambda: fill_proj(1, "k"),
                lambda: fill_lnt(1, "v"),
                lambda: fill_projv(1, [0, 1]),
                lambda: fill_projv(1, [2, 3]),
            ]
            _xts = {}

            def fill_lnt(b, tname):
                _xts[(b, tname)] = emit_lnt(b, tname, xn_on_act=False)

            def fill_proj(b, tname):
                emit_projqk(b, tname, _xts.pop((b, tname)))

            def fill_projv(b, mc_list):
                emit_projv(b, _xts[(b, "v")] if (b, "v") in _xts else None, mc_list)

            def fill_projv(b, mc_list):  # noqa: F811
                emit_projv(b, _xts[(b, "v")], mc_list)
                if mc_list[-1] == 3:
                    _xts.pop((b, "v"))

            for pr in range(H // 2):
                flush_stores()
                emit_attn_pair(0, pr, filler=fillers_b1[pr])

            fillers_o0 = [
                (lambda m=m, n=n: emit_outproj_half(0, m, n))
                for m in range(MC) for n in range(2)
            ]
            for pr in range(H // 2):
                flush_stores()
                emit_attn_pair(1, pr, filler=fillers_o0[pr])
            flush_stores()
            emit_outproj(1, [0, 1, 2, 3])

    return nc


def get_program():
    global _PROG
    if _PROG is None:
        _PROG = _build_program()
    return _PROG


def _host_prep(query, key, value, mask, q_ln_g, q_ln_b, k_ln_g, k_ln_b,
               v_ln_g, v_ln_b, Wq, bq, Wk, bk, Wv, bv, Wo, bo):
    """Fold LN affine into projection weights; pre-transpose; cast to bf16."""
    f = np.float32
    Aq = np.ascontiguousarray((Wq.astype(f) * q_ln_g.astype(f)[None, :]).T).astype(BF16)
    Ak = np.ascontiguousarray((Wk.astype(f) * k_ln_g.astype(f)[None, :]).T).astype(BF16)
    Av = np.ascontiguousarray((Wv.astype(f) * v_ln_g.astype(f)[None, :]).T).astype(BF16)
    Ao = np.ascontiguousarray(Wo.astype(f).T).astype(BF16)
    cq = (bq.astype(f) + Wq.astype(f) @ q_ln_b.astype(f)).astype(f)
    ck = (bk.astype(f) + Wk.astype(f) @ k_ln_b.astype(f)).astype(f)
    cv = (bv.astype(f) + Wv.astype(f) @ v_ln_b.astype(f)).astype(f)
    co = bo.astype(f)
    return Aq, Ak, Av, Ao, cq, ck, cv, co


def kernel(query, key, value, mask, q_ln_g, q_ln_b, k_ln_g, k_ln_b,
           v_ln_g, v_ln_b, Wq, bq, Wk, bk, Wv, bv, Wo, bo):
    global LAST_EXEC_NS, LAST_RESULTS
    from concourse.bass_utils import run_bass_kernel_spmd

    nc = get_program()
    Aq, Ak, Av, Ao, cq, ck, cv, co = _host_prep(
        query, key, value, mask, q_ln_g, q_ln_b, k_ln_g, k_ln_b,
        v_ln_g, v_ln_b, Wq, bq, Wk, bk, Wv, bv, Wo, bo,
    )
    query = np.ascontiguousarray(np.asarray(query, np.float32))
    key = np.ascontiguousarray(np.asarray(key, np.float32))
    value = np.ascontiguousarray(np.asarray(value, np.float32))
    mask = np.ascontiguousarray(np.asarray(mask, np.int32))

    in_maps = []
    for c in range(N_CORES):
        sl = slice(c * NB, (c + 1) * NB)
        in_maps.append({
            "xq": query[sl], "xk": key[sl], "xv": value[sl], "msk": mask[sl],
            "aq": Aq, "ak": Ak, "av": Av, "ao": Ao,
            "cq": cq, "ck": ck, "cv": cv, "co": co,
        })

    trace = bool(int(os.environ.get("ATTN_TRACE", "0")))
    tmpdir = os.environ.get("ATTN_TRACE_DIR") or None
    res = run_bass_kernel_spmd(
        nc, in_maps, list(range(N_CORES)), trace=trace, tmpdir=tmpdir
    )
    LAST_EXEC_NS = res.exec_time_ns
    LAST_RESULTS = res

    out = np.concatenate([r["out"] for r in res.results], axis=0)
    attnT = np.concatenate([r["attnT"] for r in res.results], axis=0)
    attn = np.ascontiguousarray(attnT.transpose(0, 1, 3, 2).astype(np.float32))
    return out, attn


# revision 42
# speedup vs baseline: 1.0523x; 1.0334x over previous
"""CrossModalAttention Trainium2 kernel (8-core SPMD, data-parallel over batch).

Self-contained: hardcodes problem shapes (B=16, Tq=Tk=512, D=1024, H=16, hd=64).
kernel(**inputs) takes the full unsharded inputs, shards batches 2-per-core
across 8 NeuronCores, runs a Bass/Tile kernel, and gathers full outputs
(out, attn) matching the reference.
"""

import math
import os
import sys

sys.path.insert(0, "/opt/trn_rl_repo")

import numpy as np
import ml_dtypes

B, T, D, H, HD = 16, 512, 1024, 16, 64
N_CORES = 8
NB = B // N_CORES          # batches per core
KC = D // 128              # feature chunks
MC = T // 128              # token chunks
EPS = 1e-5
NEG = -10000.0
SCALE = 1.0 / math.sqrt(HD)
BF16 = ml_dtypes.bfloat16

MAX_WAITS = 4  # walrus CoreV3 sync-wait slots per instruction

_PROG = None
LAST_EXEC_NS = None
LAST_RESULTS = None


def _patched_tile_context(tile_mod, bass_rust):
    """TileContext adjusted for this walrus build, which allows at most ONE
    sem wait (plus one update) per TPB instruction: excess waits are spilled
    onto standalone EventSemaphore instructions emitted just before the
    over-subscribed instruction on the same engine."""
    ScopedClock = bass_rust.ScopedClock

    def _make_wait_carrier(nc, engine, wait, tag):
        ev = bass_rust.InstEventSemaphore(
            name=f"EVW-{nc.next_id()}-{tag}", ins=[], outs=[]
        )
        ev.engine = engine
        ev.sync_info = bass_rust.SyncInfo(on_wait=[wait], on_update=[])
        return ev

    class TC(tile_mod.TileContext):
        def _lower_ordered_insts(self, ordered):
            nc = self.nc
            for bbname in list(ordered.keys()):
                insts = ordered[bbname]
                new = []
                for inst in insts:
                    si = inst.sync_info
                    waits = list(si.on_wait) if si and si.on_wait else []
                    if len(waits) > 1:
                        for w in waits[:-1]:
                            new.append(
                                _make_wait_carrier(nc, inst.engine, w, inst.name)
                            )
                        inst.sync_info = bass_rust.SyncInfo(
                            on_wait=waits[-1:],
                            on_update=list(si.on_update) if si.on_update else [],
                        )
                    new.append(inst)
                ordered[bbname] = new
            super()._lower_ordered_insts(ordered)

        def _drain_and_barrier(self, tick_clock, wait_clock):
            nc = self.nc
            drain_inst = nc.sync.drain()
            wait_clock.add_sem_waits(
                drain_inst.ins, ScopedClock({None: tick_clock.global_clock})
            )
            si = drain_inst.ins.sync_info
            waits = list(si.on_wait) if si is not None and si.on_wait else []
            if len(waits) > 1:
                drain_inst.ins.sync_info = bass_rust.SyncInfo(
                    on_wait=waits[:1],
                    on_update=list(si.on_update) if si.on_update else [],
                )
                for w in waits[1:]:
                    ev = _make_wait_carrier(nc, drain_inst.ins.engine, w, "drain")
                    nc.engines[ev.engine].add_instruction(ev)
            nc.all_engine_barrier()
            popped = nc._tile_sem_poison_stack.pop()
            assert popped is self._sem_poison
            nc.clear_and_free_semaphores(list(self.sems.allocated().values()))
            nc.all_engine_barrier()

    return TC


def _build_program():
    import concourse.bass as bass
    import concourse.mybir as mybir
    import concourse.tile as tile_mod
    import bass_rust
    from concourse.masks import make_identity

    dt = mybir.dt
    f32, bf16, i32 = dt.float32, dt.bfloat16, dt.int32
    f32r = dt.float32r
    AF = mybir.ActivationFunctionType
    OP = mybir.AluOpType

    nc = bass.Bass("TRN2", target_bir_lowering=False)

    # ---- DRAM I/O (per-core shard) ----
    xq_d = nc.dram_tensor("xq", [NB, T, D], f32, kind="ExternalInput")
    xk_d = nc.dram_tensor("xk", [NB, T, D], f32, kind="ExternalInput")
    xv_d = nc.dram_tensor("xv", [NB, T, D], f32, kind="ExternalInput")
    msk_d = nc.dram_tensor("msk", [NB, T], i32, kind="ExternalInput")
    aq_d = nc.dram_tensor("aq", [D, D], bf16, kind="ExternalInput")
    ak_d = nc.dram_tensor("ak", [D, D], bf16, kind="ExternalInput")
    av_d = nc.dram_tensor("av", [D, D], bf16, kind="ExternalInput")
    ao_d = nc.dram_tensor("ao", [D, D], bf16, kind="ExternalInput")
    cq_d = nc.dram_tensor("cq", [D], f32, kind="ExternalInput")
    ck_d = nc.dram_tensor("ck", [D], f32, kind="ExternalInput")
    cv_d = nc.dram_tensor("cv", [D], f32, kind="ExternalInput")
    co_d = nc.dram_tensor("co", [D], f32, kind="ExternalInput")
    out_d = nc.dram_tensor("out", [NB, T, D], f32, kind="ExternalOutput")
    # attn stored transposed+bf16: [b, h, k, q]; host casts + swaps last axes
    atT_d = nc.dram_tensor("attnT", [NB, H, T, T], bf16, kind="ExternalOutput")
    src_map = {"q": xq_d, "k": xk_d, "v": xv_d}

    TC = _patched_tile_context(tile_mod, bass_rust)

    with TC(nc) as tc:
        with (
            tc.tile_pool(name="wpool", bufs=1) as wpool,
            tc.tile_pool(name="xpool", bufs=3) as xpool,
            tc.tile_pool(name="xnpool", bufs=4) as xnpool,
            tc.tile_pool(name="stpool", bufs=6) as stpool,
            tc.tile_pool(name="xtpool", bufs=2) as xtpool,
            tc.tile_pool(name="qkpool", bufs=2) as qkpool,
            tc.tile_pool(name="vpool", bufs=2) as vpool,
            tc.tile_pool(name="ctpool", bufs=2) as ctpool,
            tc.tile_pool(name="epool", bufs=14) as epool,
            tc.tile_pool(name="zpool", bufs=1) as zpool,
            tc.tile_pool(name="zspool", bufs=4) as zspool,
            tc.tile_pool(name="apool", bufs=8) as apool,
            tc.tile_pool(name="opool", bufs=3) as opool,
            tc.tile_pool(name="mpool", bufs=4) as mpool,
            tc.tile_pool(name="psum", bufs=1, space="PSUM") as psum,
        ):
            # ---- prefetch batch0 'q' activations so LN starts immediately
            # (the sync DMA queues are in-order; weights would block them) ----
            prefetched = {}
            pf = []
            for mc_ in range(MC - 1):
                x_t = xpool.tile([128, D], f32, tag="x", name=f"xpf{mc_}")
                nc.sync.dma_start(out=x_t, in_=xq_d[0, mc_ * 128 : (mc_ + 1) * 128, :])
                pf.append(x_t)
            prefetched[(0, "q")] = pf

            # ---- constants / weights (tiles now; DMAs deferred until after
            # the prologue LN+transpose so activations own the queue first) ----
            aq_s = wpool.tile([128, KC, D], bf16)
            ak_s = wpool.tile([128, KC, D], bf16)
            av_s = wpool.tile([128, KC, D], bf16)
            ao_s = wpool.tile([128, KC, D], bf16)
            cq_s = wpool.tile([128, KC], f32)
            ck_s = wpool.tile([128, KC], f32)

            def emit_weight_dmas():
                nc.sync.dma_start(out=cq_s, in_=cq_d[:].rearrange("(c p) -> p c", p=128))
                aq_r = aq_d[:, :].rearrange("(kc p) n -> p kc n", p=128)
                for qtr in range(4):
                    nc.sync.dma_start(
                        out=aq_s[:, 2 * qtr : 2 * qtr + 2, :],
                        in_=aq_r[:, 2 * qtr : 2 * qtr + 2, :],
                    )
                nc.gpsimd.dma_start(out=ak_s, in_=ak_d[:, :].rearrange("(kc p) n -> p kc n", p=128))
                nc.gpsimd.dma_start(out=av_s, in_=av_d[:, :].rearrange("(kc p) n -> p kc n", p=128))
                nc.gpsimd.dma_start(out=ao_s, in_=ao_d[:, :].rearrange("(kc p) n -> p kc n", p=128))
                nc.sync.dma_start(out=ck_s, in_=ck_d[:].rearrange("(c p) -> p c", p=128))
            cv_rep = wpool.tile([128, D], bf16)
            nc.gpsimd.dma_start(
                out=cv_rep, in_=bass.AP(tensor=cv_d, offset=0, ap=[[0, 128], [1, D]])
            )
            co_rep = wpool.tile([128, D], bf16)
            nc.gpsimd.dma_start(
                out=co_rep, in_=bass.AP(tensor=co_d, offset=0, ap=[[0, 128], [1, D]])
            )
            idn = wpool.tile([128, 128], bf16)
            make_identity(nc, idn)
            ones_col = wpool.tile([1, 128], f32r)
            nc.vector.memset(ones_col.bitcast(f32), 1.0)
            eps_t = wpool.tile([128, 1], f32)
            nc.vector.memset(eps_t, EPS)

            st_ = {}  # per-batch live tiles: mb, qt, kt, vt, ct

            def emit_mask(b):
                mi = mpool.tile([128, MC], i32, name=f"mi{b}")
                nc.sync.dma_start(out=mi, in_=msk_d[b, :].rearrange("(c p) -> p c", p=128))
                mf = mpool.tile([128, MC], f32, name=f"mf{b}")
                nc.vector.tensor_copy(out=mf, in_=mi)
                mb_t = mpool.tile([128, MC], f32, name=f"mb{b}")
                nc.vector.tensor_scalar(
                    out=mb_t, in0=mf, scalar1=-NEG, scalar2=NEG, op0=OP.mult, op1=OP.add
                )
                st_[b] = {"mb": mb_t}

            def emit_lnt(b, tname, xn_on_act):
                """LN + PE-transpose one input tensor -> feature-major bf16."""
                xn_tiles = []
                pre = prefetched.pop((b, tname), None)
                for mc_ in range(MC):
                    if pre is not None and mc_ < len(pre):
                        x_t = pre[mc_]
                    else:
                        x_t = xpool.tile([128, D], f32, tag="x", name="x")
                        nc.scalar.dma_start(
                            out=x_t, in_=src_map[tname][b, mc_ * 128 : (mc_ + 1) * 128, :]
                        )
                    stt = stpool.tile([128, 2, 6], f32, tag="st", name="stt")
                    nc.vector.bn_stats(out=stt[:, 0, :], in_=x_t[:, 0:512])
                    nc.vector.bn_stats(out=stt[:, 1, :], in_=x_t[:, 512:1024])
                    mv = stpool.tile([128, 2], f32, tag="mv", name="mv")
                    nc.vector.bn_aggr(out=mv, in_=stt)
                    # rstd = exp(-0.5 * ln(var + eps)) (stays in exp table set)
                    lnv = stpool.tile([128, 1], f32, tag="lnv", name="lnv")
                    nc.scalar.activation(
                        out=lnv, in_=mv[:, 1:2], func=AF.Ln, bias=eps_t, scale=1.0
                    )
                    r_t = stpool.tile([128, 1], f32, tag="r", name="r_t")
                    nc.scalar.activation(out=r_t, in_=lnv, func=AF.Exp, scale=-0.5)
                    xn_t = xnpool.tile([128, D], bf16, tag="xn", name="xn")
                    if xn_on_act:
                        nmu = stpool.tile([128, 1], f32, tag="nmu", name="nmu")
                        nc.vector.scalar_tensor_tensor(
                            out=nmu, in0=mv[:, 0:1], scalar=-1.0, in1=r_t,
                            op0=OP.mult, op1=OP.mult,
                        )
                        nc.scalar.activation(
                            out=xn_t, in_=x_t, func=AF.Identity, bias=nmu, scale=r_t
                        )
                    else:
                        nmu = stpool.tile([128, 1], f32, tag="nmu", name="nmu")
                        nc.vector.tensor_scalar_mul(out=nmu, in0=mv[:, 0:1], scalar1=-1.0)
                        nc.vector.tensor_scalar(
                            out=xn_t, in0=x_t, scalar1=nmu, scalar2=r_t,
                            op0=OP.add, op1=OP.mult,
                        )
                    xn_tiles.append(xn_t)
                xt = xtpool.tile([128, KC, T], bf16, tag="xt", name=f"xt{b}{tname}")
                for kc_ in range(KC):
                    tp = psum.tile([128, 512], bf16, tag="sc", bufs=3, name="tp")
                    for mc_ in range(MC):
                        nc.tensor.transpose(
                            out=tp[:, mc_ * 128 : (mc_ + 1) * 128],
                            in_=xn_tiles[mc_][:, kc_ * 128 : (kc_ + 1) * 128],
                            identity=idn,
                        )
                    nc.scalar.copy(out=xt[:, kc_, :], in_=tp)
                return xt

            def emit_projqk(b, tname, xt):
                w_s, c_s = (aq_s, cq_s) if tname == "q" else (ak_s, ck_s)
                dst = qkpool.tile(
                    [128, KC, T], bf16, tag="qt" if tname == "q" else "kt",
                    name=f"{tname}t{b}",
                )
                st_[b]["qt" if tname == "q" else "kt"] = dst
                for nch in range(KC):
                    pj = psum.tile([128, 512], f32, tag="pj", bufs=3, name="pj")
                    for kc_ in range(KC):
                        nc.tensor.matmul(
                            pj,
                            lhsT=w_s[:, kc_, nch * 128 : (nch + 1) * 128],
                            rhs=xt[:, kc_, :],
                            start=(kc_ == 0),
                            stop=(kc_ == KC - 1),
                        )
                    nc.vector.tensor_scalar_add(
                        out=dst[:, nch, :], in0=pj, scalar1=c_s[:, nch : nch + 1]
                    )

            def emit_projv(b, xt, mc_list):
                if "vt" not in st_[b]:
                    vt = vpool.tile([128, MC, H, HD + 1], bf16, tag="vt", name=f"vt{b}")
                    nc.vector.memset(vt[:, :, :, HD : HD + 1], 1.0)
                    st_[b]["vt"] = vt
                vt = st_[b]["vt"]
                for mc_ in mc_list:
                    for nh in range(2):
                        pj = psum.tile([128, 512], f32, tag="pj", bufs=3, name="pj")
                        for kc_ in range(KC):
                            nc.tensor.matmul(
                                pj,
                                lhsT=xt[:, kc_, mc_ * 128 : (mc_ + 1) * 128],
                                rhs=av_s[:, kc_, nh * 512 : (nh + 1) * 512],
                                start=(kc_ == 0),
                                stop=(kc_ == KC - 1),
                            )
                        nc.vector.tensor_tensor(
                            out=vt[:, mc_, nh * 8 : (nh + 1) * 8, 0:HD],
                            in0=pj.rearrange("p (h d) -> p h d", d=HD),
                            in1=cv_rep[:, nh * 512 : (nh + 1) * 512].rearrange(
                                "p (h d) -> p h d", d=HD
                            ),
                            op=OP.add,
                        )

            pending_norms = []

            def flush_stores():
                # previous pair's zs/e tiles are ready; normalize + store now,
                # off the inter-pair critical path
                for dve_side, e_t, zs, dst in pending_norms:
                    at = apool.tile([128, 512], bf16, tag="at", name="at")
                    eng = nc.vector if dve_side else nc.gpsimd
                    eng.tensor_tensor(out=at, in0=e_t, in1=zs, op=OP.mult)
                    nc.sync.dma_start(out=dst, in_=at)
                pending_norms.clear()

            def emit_attn_pair(b, pr, filler=None):
                """Two heads; STs+exps first, then filler (next batch's PE
                work) runs while ACT exps, then AVs find exps done."""
                qt, kt, vt, mb_t = st_[b]["qt"], st_[b]["kt"], st_[b]["vt"], st_[b]["mb"]
                if "ct" not in st_[b]:
                    st_[b]["ct"] = ctpool.tile([128, KC, T], bf16, tag="ct", name=f"ct{b}")
                ct = st_[b]["ct"]
                ctzs = [
                    psum.tile([HD + 1, 512], f32, tag="ctz", bufs=2, name="ctza"),
                    psum.tile([HD + 1, 512], f32, tag="ctz", bufs=2, name="ctzb"),
                ]
                e_ts = [[], []]
                for c in range(MC):
                    stps = []
                    for hh in range(2):
                        off = hh * HD
                        stp = psum.tile([128, 512], f32, tag="sc", bufs=3, name="stp")
                        nc.tensor.matmul(
                            stp,
                            lhsT=kt[off : off + HD, pr, c * 128 : (c + 1) * 128],
                            rhs=qt[off : off + HD, pr, :],
                            start=True,
                            stop=True,
                            tile_position=(off, 0),
                        )
                        stps.append(stp)
                    for hh in range(2):
                        e_t = epool.tile([128, 512], bf16, tag="e", name="e")
                        nc.scalar.activation(
                            out=e_t, in_=stps[hh], func=AF.Exp,
                            bias=mb_t[:, c : c + 1], scale=SCALE,
                        )
                        e_ts[hh].append(e_t)
                if filler is not None:
                    filler()
                for c in range(MC):
                    for hh in range(2):
                        nc.tensor.matmul(
                            ctzs[hh], lhsT=vt[:, c, 2 * pr + hh, :], rhs=e_ts[hh][c],
                            start=(c == 0), stop=(c == MC - 1),
                        )
                for hh in range(2):
                    h = 2 * pr + hh
                    off = hh * HD
                    ctz = ctzs[hh]
                    # 1/Z = exp(-ln(Z)) on ACT (same table set as Exp)
                    zln = zpool.tile([1, 512], f32, tag="zln", name="zln")
                    nc.scalar.activation(out=zln, in_=ctz[HD : HD + 1, :], func=AF.Ln)
                    rz = zpool.tile([1, 512], f32r, tag="rz", name="rz")
                    nc.scalar.activation(out=rz, in_=zln, func=AF.Exp, scale=-1.0)
                    zrep = psum.tile([128, 512], f32, tag="sc", bufs=3, name="zrep")
                    nc.tensor.matmul(zrep, lhsT=ones_col, rhs=rz, start=True, stop=True)
                    zs = zspool.tile([128, 512], bf16, tag="zs", name="zs")
                    nc.vector.tensor_copy(out=zs, in_=zrep)
                    nc.vector.tensor_tensor(
                        out=ct[off : off + HD, pr, :], in0=ctz[0:HD, :],
                        in1=zs[0:HD, :], op=OP.mult,
                    )
                    for c in range(MC):
                        dve_side = (c % 2 == hh % 2)
                        pending_norms.append(
                            (dve_side, e_ts[hh][c], zs,
                             atT_d[b, h, c * 128 : (c + 1) * 128, :])
                        )

            def emit_outproj_half(b, mc_, nh):
                ct = st_[b]["ct"]
                pj = psum.tile([128, 512], f32, tag="pj", bufs=3, name="pjo")
                for j in range(KC):
                    nc.tensor.matmul(
                        pj,
                        lhsT=ct[:, j, mc_ * 128 : (mc_ + 1) * 128],
                        rhs=ao_s[:, j, nh * 512 : (nh + 1) * 512],
                        start=(j == 0),
                        stop=(j == KC - 1),
                    )
                ot = opool.tile([128, 512], f32, tag="ot", name="ot")
                nc.vector.tensor_tensor(
                    out=ot, in0=pj,
                    in1=co_rep[:, nh * 512 : (nh + 1) * 512], op=OP.add,
                )
                qeng = nc.scalar if b == 1 else nc.sync
                qeng.dma_start(
                    out=out_d[b, mc_ * 128 : (mc_ + 1) * 128,
                              nh * 512 : (nh + 1) * 512],
                    in_=ot,
                )

            def emit_outproj(b, mc_list):
                for mc_ in mc_list:
                    for nh in range(2):
                        emit_outproj_half(b, mc_, nh)

            # ---- schedule: prologue batch0, then attn(b) interleaved with
            # batch b+1 projection pipeline / batch b-1 out-projection ----
            emit_mask(0)
            xt_q0 = emit_lnt(0, "q", xn_on_act=True)
            emit_weight_dmas()
            emit_projqk(0, "q", xt_q0)
            xt_k0 = emit_lnt(0, "k", xn_on_act=True)
            emit_projqk(0, "k", xt_k0)
            xt_v0 = emit_lnt(0, "v", xn_on_act=True)
            emit_projv(0, xt_v0, [0, 1, 2, 3])

            fillers_b1 = [
                lambda: emit_mask(1),
                lambda: fill_lnt(1, "q"),
                lambda: fill_proj(1, "q"),
                lambda: fill_lnt(1, "k"),
                lambda: fill_proj(1, "k"),
                lambda: fill_lnt(1, "v"),
                lambda: fill_projv(1, [0, 1]),
                lambda: fill_projv(1, [2, 3]),
            ]
            _xts = {}

            def fill_lnt(b, tname):
                _xts[(b, tname)] = emit_lnt(b, tname, xn_on_act=False)

            def fill_proj(b, tname):
                emit_projqk(b, tname, _xts.pop((b, tname)))

            def fill_projv(b, mc_list):
                emit_projv(b, _xts[(b, "v")] if (b, "v") in _xts else None, mc_list)

            def fill_projv(b, mc_list):  # noqa: F811
                emit_projv(b, _xts[(b, "v")], mc_list)
                if mc_list[-1] == 3:
                    _xts.pop((b, "v"))

            for pr in range(H // 2):
                flush_stores()
                emit_attn_pair(0, pr, filler=fillers_b1[pr])

            fillers_o0 = [
                (lambda m=m, n=n: emit_outproj_half(0, m, n))
                for m in range(MC) for n in range(2)
            ]
            for pr in range(H // 2):
                flush_stores()
                emit_attn_pair(1, pr, filler=fillers_o0[pr])
            flush_stores()
            emit_outproj(1, [0, 1, 2, 3])

    return nc


def get_program():
    global _PROG
    if _PROG is None:
        _PROG = _build_program()
    return _PROG


def _host_prep(query, key, value, mask, q_ln_g, q_ln_b, k_ln_g, k_ln_b,
               v_ln_g, v_ln_b, Wq, bq, Wk, bk, Wv, bv, Wo, bo):
    """Fold LN affine into projection weights; pre-transpose; cast to bf16."""
    f = np.float32
    Aq = np.ascontiguousarray((Wq.astype(f) * q_ln_g.astype(f)[None, :]).T).astype(BF16)
    Ak = np.ascontiguousarray((Wk.astype(f) * k_ln_g.astype(f)[None, :]).T).astype(BF16)
    Av = np.ascontiguousarray((Wv.astype(f) * v_ln_g.astype(f)[None, :]).T).astype(BF16)
    Ao = np.ascontiguousarray(Wo.astype(f).T).astype(BF16)
    cq = (bq.astype(f) + Wq.astype(f) @ q_ln_b.astype(f)).astype(f)
    ck = (bk.astype(f) + Wk.astype(f) @ k_ln_b.astype(f)).astype(f)
    cv = (bv.astype(f) + Wv.astype(f) @ v_ln_b.astype(f)).astype(f)
    co = bo.astype(f)
    return Aq, Ak, Av, Ao, cq, ck, cv, co


def kernel(query, key, value, mask, q_ln_g, q_ln_b, k_ln_g, k_ln_b,
           v_ln_g, v_ln_b, Wq, bq, Wk, bk, Wv, bv, Wo, bo):
    global LAST_EXEC_NS, LAST_RESULTS
    from concourse.bass_utils import run_bass_kernel_spmd

    nc = get_program()
    Aq, Ak, Av, Ao, cq, ck, cv, co = _host_prep(
        query, key, value, mask, q_ln_g, q_ln_b, k_ln_g, k_ln_b,
        v_ln_g, v_ln_b, Wq, bq, Wk, bk, Wv, bv, Wo, bo,
    )
    query = np.ascontiguousarray(np.asarray(query, np.float32))
    key = np.ascontiguousarray(np.asarray(key, np.float32))
    value = np.ascontiguousarray(np.asarray(value, np.float32))
    mask = np.ascontiguousarray(np.asarray(mask, np.int32))

    in_maps = []
    for c in range(N_CORES):
        sl = slice(c * NB, (c + 1) * NB)
        in_maps.append({
            "xq": query[sl], "xk": key[sl], "xv": value[sl], "msk": mask[sl],
            "aq": Aq, "ak": Ak, "av": Av, "ao": Ao,
            "cq": cq, "ck": ck, "cv": cv, "co": co,
        })

    trace = bool(int(os.environ.get("ATTN_TRACE", "0")))
    tmpdir = os.environ.get("ATTN_TRACE_DIR") or None
    res = run_bass_kernel_spmd(
        nc, in_maps, list(range(N_CORES)), trace=trace, tmpdir=tmpdir
    )
    LAST_EXEC_NS = res.exec_time_ns
    LAST_RESULTS = res

    out = np.concatenate([r["out"] for r in res.results], axis=0)
    attnT = np.concatenate([r["attnT"] for r in res.results], axis=0)
    attn = np.ascontiguousarray(attnT.transpose(0, 1, 3, 2).astype(np.float32))
    return out, attn
